# revision 34
# baseline (speedup 1.0000x reference)
"""Trainium2 Bass kernel for batched additive-attention scoring.

Computes, for each batch b:
    out[b] = softmax_s( sum_h v[h] * tanh( (W1 @ static[b])[h,s]
                                         + (W2 @ dynamic[b])[h,s]
                                         + (W3 @ hidden[b])[h] ) )

Sharding: data-parallel over batch B=64 across 8 NeuronCores (8 batches
per core); small params (W, v) replicated.  No collectives needed.

Per-core dataflow (H=256, S=4096):
  - encoders are cast to fp16 and concatenated on host: 32 MB/core of
    DMA (f32 would be 64 MB, ~179 us at ~358 GB/s); fp16 matmuls run at
    the same 1 cycle/row as f32r (measured 216 ns steady-state spacing
    at N=512, LDWEIGHTS fully pulled ahead); total quantization error
    ~9e-4 vs the 2e-2 gate.  Full [128, 4096] chunk DMAs keep 8 KB
    descriptors (4 KB descriptors measured only ~280 GB/s); batch 0
    streams each chunk in 4 quarter-DMAs into one tile (subtile deps)
    so the PE starts ~3 us in.
  - per (batch, s-pair): 8 E-matmuls (N=512, the PSUM-bank ISA cap)
    accumulate K=512 into the two halves of a [128, 2, 512] two-bank
    psum tile; ONE tanh activation reads the flat [128, 1024] view
    (per-partition bias, fp16 out) — halves ACT's per-instruction
    fixed overhead (measured ~684 ns per 512-elem activation).
  - the two v-matmuls of each group run TWO GROUPS LATE (software
    pipelining) so they never sit in PE's queue waiting on tanh —
    measured ~100-130 ns/matmul of sem-wait stall otherwise.
  - scores accumulate in TWO psum banks (batches 0-3 | 4-7), row
    8b+jj = (batch b, s-tile jj) as row jj%32 of its half.  lhsT is a
    sliding 32-wide window into vt_ext [128, 2, 63] (v chunk
    zero-padded both sides) so v lands in exactly its row's column and
    every other row gets +0.
  - per half: one Exp [32,512] with accum_out; per-batch totals via a
    zero-padded block-diag ones fp16 matmul (small-shape f32r matmuls
    fail walrus ISA checks); DVE reciprocal; broadcast back via the
    transposed ones matmul; DVE scale; 64 KB output DMA.  Half A runs
    mid-kernel (hidden behind batches 4-7); only half B sits on the
    tail (~11 us incl. the fixed ~8 us engine-drain teardown).
"""

import os
import sys
from contextlib import ExitStack

import numpy as np

for _p in ("/root/.axon_site", "/root/.axon_site/_ro/trn_rl_repo",
           "/root/.axon_site/_ro/pypackages", "/opt/trn_rl_repo", "/opt/pypackages"):
    if os.path.isdir(_p) and _p not in sys.path:
        sys.path.append(_p)

import concourse.bass as bass
import concourse.tile as tile
from concourse import bacc, mybir
from concourse._compat import with_exitstack
from concourse.bass_utils import run_bass_kernel_spmd

H = 256
S = 4096
B = 64
NCORES = 8
BPC = B // NCORES  # batches per core

F32 = mybir.dt.float32
F32R = mybir.dt.float32r
F16 = mybir.dt.float16
TANH = mybir.ActivationFunctionType.Tanh
EXP = mybir.ActivationFunctionType.Exp

ST = 512           # matmul output tile (one PSUM bank of f32, ISA cap)
NS = S // ST       # 8 s-tiles
NG = NS // 2       # 4 s-pairs per batch
NM = H // 128      # 2 m-blocks (output h partition blocks)
NK = (2 * H) // 128  # 4 k-chunks of the concatenated [static; dynamic]
NROW = BPC * NS    # 64 score rows (one per (batch, s-tile))
HROW = NROW // 2   # rows per scores half-bank


@with_exitstack
def _attn_kernel(ctx: ExitStack, tc: "tile.TileContext",
                 out_ap, x_ap, wt_ap, w3t_ap, vt_ap, ht_ap, blk_ap, blkT_ap):
    nc = tc.nc

    const = ctx.enter_context(tc.tile_pool(name="const", bufs=1))
    xpool = ctx.enter_context(tc.tile_pool(name="x", bufs=14))
    epsum = ctx.enter_context(tc.tile_pool(name="epsum", bufs=3, space="PSUM"))
    scpsum = ctx.enter_context(tc.tile_pool(name="scpsum", bufs=1, space="PSUM"))
    esb = ctx.enter_context(tc.tile_pool(name="esb", bufs=10))
    rows = ctx.enter_context(tc.tile_pool(name="rows", bufs=1))
    tiny = ctx.enter_context(tc.tile_pool(name="tiny", bufs=4))

    # ---- head: get batch 0's first s-slice and wt in flight before anything
    # else (each DMA issue costs ~650 ns on its ring and ~2 us completion
    # latency; the first E-matmul gates on wt + the q0 slice of all 4
    # chunks), all on the sync HWDGE ring (SWDGE completion is slower).
    Q0 = 1024
    wt_sb = const.tile([128, NK, H], F16)        # [p, kchunk, h]
    xt0 = []
    for c in range(NK):
        t = xpool.tile([128, S], F16, tag="x", name=f"x{c}")
        xt0.append(t)
    # interleave per-chunk wt slices with the q0 x-slices so each chunk's
    # weights land just before its first matmul instead of the whole wt
    # transfer (256 KB) serializing ahead of all x data
    nc.sync.dma_start(wt_sb[:, 0, :], wt_ap[:, 0, :])
    nc.sync.dma_start(xt0[0][:, 0:Q0], x_ap[0, 0:128, 0:Q0])
    nc.sync.dma_start(xt0[1][:, 0:Q0], x_ap[0, 128:256, 0:Q0])
    nc.sync.dma_start(wt_sb[:, 1, :], wt_ap[:, 1, :])
    nc.sync.dma_start(xt0[2][:, 0:Q0], x_ap[0, 256:384, 0:Q0])
    nc.sync.dma_start(wt_sb[:, 2, :], wt_ap[:, 2, :])
    nc.sync.dma_start(xt0[3][:, 0:Q0], x_ap[0, 384:512, 0:Q0])
    nc.sync.dma_start(wt_sb[:, 3, :], wt_ap[:, 3, :])

    # remaining replicated params (bias inputs first: needed ~15 us in)
    w3_sb = const.tile([128, 2, H], F32R)        # [p, kchunk, h]
    nc.gpsimd.dma_start(w3_sb[:], w3t_ap)
    ht_sb = const.tile([128, 2, BPC], F32R)      # [p, kchunk, b]
    nc.gpsimd.dma_start(ht_sb[:], ht_ap)
    vt_sb = const.tile([128, 2, 2 * HROW - 1], F16)  # [p, hchunk, padded col]
    nc.gpsimd.dma_start(vt_sb[:], vt_ap)
    blk_sb = const.tile([128, BPC], F16)         # block-diag ones, 0-padded
    nc.gpsimd.dma_start(blk_sb[:], blk_ap)
    blkT_sb = const.tile([128, NROW], F16)       # its transpose, 0-padded
    nc.gpsimd.dma_start(blkT_sb[:], blkT_ap)
    for q in range(1, S // Q0):  # rest of batch 0, c-major interleaved
        for c in range(NK):
            nc.sync.dma_start(xt0[c][:, q * Q0:(q + 1) * Q0],
                              x_ap[0, c * 128:(c + 1) * 128, q * Q0:(q + 1) * Q0])

    # two psum banks accumulating score rows (batches 0-3 | 4-7) so the
    # first half's exp runs mid-kernel instead of on the tail
    scores_box = [None, None]

    # ---- bias[h, b] = sum_k W3T[k,h] * hiddenT[k,b] (all batches at once).
    # Emitted AFTER batch 0's first E-matmul group: the in-order PE
    # sequencer would otherwise stall on the (slow SWDGE) w3/ht loads
    # before dispatching any E-matmul, delaying kernel start.
    bias_sb = const.tile([128, NM, BPC], F32)  # [p, m, b]

    def emit_bias():
        for m in range(NM):
            bp = scpsum.tile([128, BPC], F32, tag="scA", name="bp")
            for c in range(2):
                nc.tensor.matmul(bp[:],
                                 lhsT=w3_sb[:, c, m * 128:(m + 1) * 128],
                                 rhs=ht_sb[:, c, :],
                                 start=(c == 0), stop=(c == 1))
            nc.vector.tensor_copy(bias_sb[:, m, :], bp[:])

    exp_sb = rows.tile([NROW, ST], F32, tag="exp")
    sums = tiny.tile([128, 2], F16, tag="sums")
    inv = tiny.tile([128, 2], F16, tag="inv")
    first_v = [True, True]

    inv32 = tiny.tile([NROW, 1], F32, tag="inv32")

    def emit_exp(h):
        # exp one half of the score rows; h=0 runs mid-kernel (hidden
        # behind batches 4-7), h=1 on the tail
        with nc.allow_low_precision(reason="fp16 denominators, ~5e-4 rel"):
            nc.scalar.activation(
                exp_sb[h * HROW:(h + 1) * HROW, :], scores_box[h][:], EXP,
                accum_out=sums[h * HROW:(h + 1) * HROW, 0:1])

    def emit_normalize(h):
        # per-batch totals (block-diag ones matmul), reciprocal, broadcast
        # back (transposed ones matmul), scale, and output DMA for one
        # half of the rows; h=0 runs hidden behind batches 4-7's compute
        p0 = h * HROW  # partition base: 0 for half A, 32 for half B
        tot = scpsum.tile([NROW, 2], F32, tag="scA", name=f"tot{h}")
        nc.tensor.matmul(tot[p0:p0 + BPC // 2, :],
                         lhsT=blk_sb[p0:p0 + HROW,
                                     h * (BPC // 2):(h + 1) * (BPC // 2)],
                         rhs=sums[p0:p0 + HROW, :], start=True, stop=True)
        with nc.allow_low_precision(reason="fp16 denominators, ~5e-4 rel"):
            nc.vector.reciprocal(inv[p0:p0 + BPC // 2, 0:1],
                                 tot[p0:p0 + BPC // 2, 0:1])
        invp = scpsum.tile([NROW, 2], F32, tag="scA", name=f"invp{h}")
        nc.tensor.matmul(invp[p0:p0 + HROW, :],
                         lhsT=blkT_sb[p0:p0 + HROW, p0:p0 + HROW],
                         rhs=inv[p0:p0 + HROW, :], start=True, stop=True)
        nc.vector.tensor_copy(inv32[p0:p0 + HROW, :], invp[p0:p0 + HROW, 0:1])
        nc.vector.tensor_scalar_mul(exp_sb[p0:p0 + HROW, :],
                                    exp_sb[p0:p0 + HROW, :],
                                    inv32[p0:p0 + HROW, :])
        nc.sync.dma_start(out_ap[h * (BPC // 2):(h + 1) * (BPC // 2), :],
                          exp_sb[p0:p0 + HROW, :])

    def emit_v(pend):
        # v-matmuls for a group whose tanh was issued two groups ago
        r2, es_pair = pend
        h, hr2 = r2 // HROW, r2 % HROW
        scores = scores_box[h]
        for j in range(2):
            r = hr2 + j
            for c in range(2):
                # v chunk c sits at column r of the sliding window
                nc.tensor.matmul(
                    scores[:],
                    lhsT=vt_sb[:, c, (HROW - 1) - r:(HROW - 1) - r + HROW],
                    rhs=es_pair[c][:, j, :],
                    start=first_v[h],
                    stop=(r == HROW - 1 and c == 1),
                    skip_group_check=True)
                first_v[h] = False
        if r2 + 2 == HROW:
            emit_exp(0)
        elif r2 == HROW:
            emit_normalize(0)

    pending = []
    for b in range(BPC):
        # stream the 4 k-chunks as full [128, 4096] DMAs (8 KB descriptor
        # rows); batch 0's were issued above.
        if b == 0:
            xt = xt0
        else:
            xt = []
            for c in range(NK):
                t = xpool.tile([128, S], F16, tag="x", name=f"x{c}")
                xt.append(t)
            for c in range(NK):
                nc.sync.dma_start(xt[c][:], x_ap[b, c * 128:(c + 1) * 128, :])

        for g in range(NG):
            es_pair = []
            for m in range(NM):
                eps = epsum.tile([128, 2, ST], F32, tag="ep")
                for c in range(NK):
                    for j in range(2):
                        nc.tensor.matmul(
                            eps[:, j, :],
                            lhsT=wt_sb[:, c, m * 128:(m + 1) * 128],
                            rhs=xt[c][:, (2 * g + j) * ST:(2 * g + j + 1) * ST],
                            start=(c == 0), stop=(c == NK - 1))
                if b == 0 and g == 0 and m == 0:
                    # bias matmuls ride here: after a group of E-matmuls
                    # (so the PE has work while the SWDGE w3/ht loads
                    # land) but before the first tanh that reads bias_sb
                    emit_bias()
                    scores_box[0] = scpsum.tile([HROW, ST], F32, tag="scA",
                                                name="scoresA")
                    scores_box[1] = scpsum.tile([HROW, ST], F32, tag="scB",
                                                name="scoresB")
                    nc.vector.memset(sums[:], 0.0)
                    nc.vector.memset(inv[:], 0.0)
                es = esb.tile([128, 2, ST], F16, tag="es")
                nc.scalar.activation(es[:], eps[:],
                                     TANH, bias=bias_sb[:, m, b:b + 1])
                es_pair.append(es)

            pending.append((b * NS + 2 * g, es_pair))
            if len(pending) > 2:
                emit_v(pending.pop(0))

    for pend in pending:
        emit_v(pend)

    # ---- tail: exp + normalize + output DMA for half B only (half A
    # already went out mid-kernel)
    emit_exp(1)
    emit_normalize(1)


_CACHED = None


def _build():
    global _CACHED
    if _CACHED is not None:
        return _CACHED
    nc = bacc.Bacc("TRN2", target_bir_lowering=False, debug=False,
                   num_devices=NCORES)
    x = nc.dram_tensor("x", (BPC, 2 * H, S), F16, kind="ExternalInput").ap()
    wt = nc.dram_tensor("wt", (128, NK, H), F16, kind="ExternalInput").ap()
    w3t = nc.dram_tensor("w3t", (128, 2, H), F32R, kind="ExternalInput").ap()
    vt = nc.dram_tensor("vt", (128, 2, 2 * HROW - 1), F16, kind="ExternalInput").ap()
    ht = nc.dram_tensor("ht", (128, 2, BPC), F32R, kind="ExternalInput").ap()
    blk = nc.dram_tensor("blk", (128, BPC), F16, kind="ExternalInput").ap()
    blkT = nc.dram_tensor("blkT", (128, NROW), F16, kind="ExternalInput").ap()
    out = nc.dram_tensor("out", (BPC, S), F32, kind="ExternalOutput").ap()

    with tile.TileContext(nc) as tc:
        _attn_kernel(tc, out, x, wt, w3t, vt, ht, blk, blkT)
    nc.compile()
    _CACHED = nc
    return nc


def _chunk_major(a: np.ndarray) -> np.ndarray:
    """[C*128, F] -> [128, C, F] so partition p holds rows {p, 128+p, ...}."""
    c = a.shape[0] // 128
    return np.ascontiguousarray(a.reshape(c, 128, -1).transpose(1, 0, 2))


def kernel(static_enc, dynamic_enc, decoder_hidden, v, W, *, _trace=False,
           **trace_kwargs):
    static_enc = np.asarray(static_enc, dtype=np.float16)
    dynamic_enc = np.asarray(dynamic_enc, dtype=np.float16)
    decoder_hidden = np.ascontiguousarray(decoder_hidden, dtype=np.float32)
    v = np.ascontiguousarray(v, dtype=np.float32)
    W = np.ascontiguousarray(W, dtype=np.float32)

    nc = _build()

    xcat = np.concatenate([static_enc, dynamic_enc], axis=1)  # [B, 2H, S]
    wt = _chunk_major(np.concatenate([W[:, :H].T, W[:, H:2 * H].T],
                                     axis=0).astype(np.float16))
    w3t = _chunk_major(np.ascontiguousarray(W[:, 2 * H:].T))
    # vt_ext[p, c, :] = [0]*31 ++ [v_c[p]] ++ [0]*31 ; lhsT window starting
    # at (HROW-1)-r puts v at output column r of its half, zeros elsewhere.
    vt_ext = np.zeros((128, 2, 2 * HROW - 1), dtype=np.float16)
    vt_ext[:, :, HROW - 1] = v.reshape(2, 128).T.astype(np.float16)
    blk = np.zeros((128, BPC), dtype=np.float16)
    for r in range(NROW):
        blk[r, r // NS] = 1.0
    # blkT[p, r] = 1 iff inv at partition p feeds score row r, per half:
    # half A uses rows/partitions 0-31, half B rows/partitions 32-63
    blkT = np.zeros((128, NROW), dtype=np.float16)
    for r in range(NROW):
        h = r // (NROW // 2)
        blkT[32 * h + (r - 32 * h) // NS, r] = 1.0
    in_maps = []
    for i in range(NCORES):
        sl = slice(i * BPC, (i + 1) * BPC)
        ht = _chunk_major(np.ascontiguousarray(decoder_hidden[sl].T))
        in_maps.append({
            "x": xcat[sl],
            "wt": wt, "w3t": w3t, "vt": vt_ext, "ht": ht,
            "blk": blk, "blkT": blkT,
        })

    res = run_bass_kernel_spmd(nc, in_maps, core_ids=list(range(NCORES)),
                               trace=_trace, **trace_kwargs)
    kernel.last_result = res
    return np.concatenate([res.results[i]["out"] for i in range(NCORES)], axis=0)


kernel.last_result = None


# revision 37
# speedup vs baseline: 1.0747x; 1.0747x over previous
"""Trainium2 Bass kernel for batched additive-attention scoring.

Computes, for each batch b:
    out[b] = softmax_s( sum_h v[h] * tanh( (W1 @ static[b])[h,s]
                                         + (W2 @ dynamic[b])[h,s]
                                         + (W3 @ hidden[b])[h] ) )

Sharding: data-parallel over batch B=64 across 8 NeuronCores (8 batches
per core); small params (W, v) replicated.  No collectives needed.

Per-core dataflow (H=256, S=4096):
  - encoders are cast to fp16 and concatenated on host: 32 MB/core of
    DMA (f32 would be 64 MB, ~179 us at ~358 GB/s); fp16 matmuls run at
    the same 1 cycle/row as f32r (measured 216 ns steady-state spacing
    at N=512, LDWEIGHTS fully pulled ahead); total quantization error
    ~9e-4 vs the 2e-2 gate.  Full [128, 4096] chunk DMAs keep 8 KB
    descriptors (4 KB descriptors measured only ~280 GB/s); batch 0
    streams each chunk in 4 quarter-DMAs into one tile (subtile deps)
    so the PE starts ~3 us in.
  - per (batch, s-pair): 8 E-matmuls (N=512, the PSUM-bank ISA cap)
    accumulate K=512 into the two halves of a [128, 2, 512] two-bank
    psum tile; ONE tanh activation reads the flat [128, 1024] view
    (per-partition bias, fp16 out) — halves ACT's per-instruction
    fixed overhead (measured ~684 ns per 512-elem activation).
  - the two v-matmuls of each group run TWO GROUPS LATE (software
    pipelining) so they never sit in PE's queue waiting on tanh —
    measured ~100-130 ns/matmul of sem-wait stall otherwise.
  - scores accumulate in TWO psum banks (batches 0-3 | 4-7), row
    8b+jj = (batch b, s-tile jj) as row jj%32 of its half.  lhsT is a
    sliding 32-wide window into vt_ext [128, 2, 63] (v chunk
    zero-padded both sides) so v lands in exactly its row's column and
    every other row gets +0.
  - per half: one Exp [32,512] with accum_out; per-batch totals via a
    zero-padded block-diag ones fp16 matmul (small-shape f32r matmuls
    fail walrus ISA checks); DVE reciprocal; broadcast back via the
    transposed ones matmul; DVE scale; 64 KB output DMA.  Half A runs
    mid-kernel (hidden behind batches 4-7); only half B sits on the
    tail (~11 us incl. the fixed ~8 us engine-drain teardown).
"""

import os
import sys
from contextlib import ExitStack

import numpy as np

for _p in ("/root/.axon_site", "/root/.axon_site/_ro/trn_rl_repo",
           "/root/.axon_site/_ro/pypackages", "/opt/trn_rl_repo", "/opt/pypackages"):
    if os.path.isdir(_p) and _p not in sys.path:
        sys.path.append(_p)

import concourse.bass as bass
import concourse.tile as tile
from concourse import bacc, mybir
from concourse._compat import with_exitstack
from concourse.bass_utils import run_bass_kernel_spmd

H = 256
S = 4096
B = 64
NCORES = 8
BPC = B // NCORES  # batches per core

F32 = mybir.dt.float32
F32R = mybir.dt.float32r
F16 = mybir.dt.float16
TANH = mybir.ActivationFunctionType.Tanh
EXP = mybir.ActivationFunctionType.Exp

ST = 512           # matmul output tile (one PSUM bank of f32, ISA cap)
NS = S // ST       # 8 s-tiles
NG = NS // 2       # 4 s-pairs per batch
NM = H // 128      # 2 m-blocks (output h partition blocks)
NK = (2 * H) // 128  # 4 k-chunks of the concatenated [static; dynamic]
NROW = BPC * NS    # 64 score rows (one per (batch, s-tile))
HROW = NROW // 2   # rows per scores half-bank


@with_exitstack
def _attn_kernel(ctx: ExitStack, tc: "tile.TileContext",
                 out_ap, x_ap, wt_ap, w3t_ap, vt_ap, ht_ap, blk_ap, blkT_ap,
                 vp_ap):
    nc = tc.nc

    const = ctx.enter_context(tc.tile_pool(name="const", bufs=1))
    xpool = ctx.enter_context(tc.tile_pool(name="x", bufs=12))
    epsum = ctx.enter_context(tc.tile_pool(name="epsum", bufs=3, space="PSUM"))
    scpsum = ctx.enter_context(tc.tile_pool(name="scpsum", bufs=1, space="PSUM"))
    esb = ctx.enter_context(tc.tile_pool(name="esb", bufs=8))
    ecb = ctx.enter_context(tc.tile_pool(name="ecb", bufs=6))
    rows = ctx.enter_context(tc.tile_pool(name="rows", bufs=1))
    tiny = ctx.enter_context(tc.tile_pool(name="tiny", bufs=4))

    # ---- head: get batch 0's first s-slice and wt in flight before anything
    # else (each DMA issue costs ~650 ns on its ring and ~2 us completion
    # latency; the first E-matmul gates on wt + the q0 slice of all 4
    # chunks), all on the sync HWDGE ring (SWDGE completion is slower).
    Q0 = 1024
    wt_sb = const.tile([128, NK, H], F16)        # [p, kchunk, h]
    xt0 = []
    for c in range(NK):
        t = xpool.tile([128, S], F16, tag="x", name=f"x{c}")
        xt0.append(t)
    # interleave per-chunk wt slices with the q0 x-slices so each chunk's
    # weights land just before its first matmul instead of the whole wt
    # transfer (256 KB) serializing ahead of all x data
    nc.sync.dma_start(wt_sb[:, 0, :], wt_ap[:, 0, :])
    nc.sync.dma_start(xt0[0][:, 0:Q0], x_ap[0, 0:128, 0:Q0])
    nc.sync.dma_start(xt0[1][:, 0:Q0], x_ap[0, 128:256, 0:Q0])
    nc.sync.dma_start(wt_sb[:, 1, :], wt_ap[:, 1, :])
    nc.sync.dma_start(xt0[2][:, 0:Q0], x_ap[0, 256:384, 0:Q0])
    nc.sync.dma_start(wt_sb[:, 2, :], wt_ap[:, 2, :])
    nc.sync.dma_start(xt0[3][:, 0:Q0], x_ap[0, 384:512, 0:Q0])
    nc.sync.dma_start(wt_sb[:, 3, :], wt_ap[:, 3, :])

    # remaining replicated params (bias inputs first: needed ~15 us in)
    w3_sb = const.tile([128, 2, H], F32R)        # [p, kchunk, h]
    nc.gpsimd.dma_start(w3_sb[:], w3t_ap)
    ht_sb = const.tile([128, 2, BPC], F32R)      # [p, kchunk, b]
    nc.gpsimd.dma_start(ht_sb[:], ht_ap)
    vt_sb = const.tile([128, 2, 2 * HROW - 1], F16)  # [p, hchunk, padded col]
    nc.gpsimd.dma_start(vt_sb[:], vt_ap)
    blk_sb = const.tile([128, BPC], F16)         # block-diag ones, 0-padded
    nc.gpsimd.dma_start(blk_sb[:], blk_ap)
    blkT_sb = const.tile([128, NROW], F16)       # its transpose, 0-padded
    nc.gpsimd.dma_start(blkT_sb[:], blkT_ap)
    vp_sb = const.tile([128, 2], F32)            # v chunks, per-partition
    nc.gpsimd.dma_start(vp_sb[:], vp_ap)
    for q in range(1, S // Q0):  # rest of batch 0, c-major interleaved
        for c in range(NK):
            nc.sync.dma_start(xt0[c][:, q * Q0:(q + 1) * Q0],
                              x_ap[0, c * 128:(c + 1) * 128, q * Q0:(q + 1) * Q0])

    # two psum banks accumulating score rows (batches 0-3 | 4-7) so the
    # first half's exp runs mid-kernel instead of on the tail
    scores_box = [None, None]

    # ---- bias[h, b] = sum_k W3T[k,h] * hiddenT[k,b] (all batches at once).
    # Emitted AFTER batch 0's first E-matmul group: the in-order PE
    # sequencer would otherwise stall on the (slow SWDGE) w3/ht loads
    # before dispatching any E-matmul, delaying kernel start.
    bias_sb = const.tile([128, NM, BPC], F32)  # [p, m, b]

    def emit_bias():
        for m in range(NM):
            bp = scpsum.tile([128, BPC], F32, tag="scA", name="bp")
            for c in range(2):
                nc.tensor.matmul(bp[:],
                                 lhsT=w3_sb[:, c, m * 128:(m + 1) * 128],
                                 rhs=ht_sb[:, c, :],
                                 start=(c == 0), stop=(c == 1))
            nc.vector.tensor_copy(bias_sb[:, m, :], bp[:])

    exp_sb = rows.tile([NROW, ST], F32, tag="exp")
    sums = tiny.tile([128, 2], F16, tag="sums")
    inv = tiny.tile([128, 2], F16, tag="inv")
    first_v = [True, True]

    inv32 = tiny.tile([NROW, 1], F32, tag="inv32")

    def emit_exp(h):
        # exp one half of the score rows; h=0 runs mid-kernel (hidden
        # behind batches 4-7), h=1 on the tail
        with nc.allow_low_precision(reason="fp16 denominators, ~5e-4 rel"):
            nc.scalar.activation(
                exp_sb[h * HROW:(h + 1) * HROW, :], scores_box[h][:], EXP,
                accum_out=sums[h * HROW:(h + 1) * HROW, 0:1])

    def emit_normalize(h):
        # per-batch totals (block-diag ones matmul), reciprocal, broadcast
        # back (transposed ones matmul), scale, and output DMA for one
        # half of the rows; h=0 runs hidden behind batches 4-7's compute
        p0 = h * HROW  # partition base: 0 for half A, 32 for half B
        tot = scpsum.tile([NROW, 2], F32, tag="scA", name=f"tot{h}")
        nc.tensor.matmul(tot[p0:p0 + BPC // 2, :],
                         lhsT=blk_sb[p0:p0 + HROW,
                                     h * (BPC // 2):(h + 1) * (BPC // 2)],
                         rhs=sums[p0:p0 + HROW, :], start=True, stop=True)
        with nc.allow_low_precision(reason="fp16 denominators, ~5e-4 rel"):
            nc.vector.reciprocal(inv[p0:p0 + BPC // 2, 0:1],
                                 tot[p0:p0 + BPC // 2, 0:1])
        invp = scpsum.tile([NROW, 2], F32, tag="scA", name=f"invp{h}")
        nc.tensor.matmul(invp[p0:p0 + HROW, :],
                         lhsT=blkT_sb[p0:p0 + HROW, p0:p0 + HROW],
                         rhs=inv[p0:p0 + HROW, :], start=True, stop=True)
        nc.vector.tensor_copy(inv32[p0:p0 + HROW, :], invp[p0:p0 + HROW, 0:1])
        nc.vector.tensor_scalar_mul(exp_sb[p0:p0 + HROW, :],
                                    exp_sb[p0:p0 + HROW, :],
                                    inv32[p0:p0 + HROW, :])
        nc.sync.dma_start(out_ap[h * (BPC // 2):(h + 1) * (BPC // 2), :],
                          exp_sb[p0:p0 + HROW, :])

    def emit_v(pend):
        # one ones-window matmul per s-tile: the v-weighting already
        # happened on the DVE (esc = v0*Eb0 + v1*Eb1), so the matmul is
        # a plain partition sum into score row r
        r2, esc = pend
        h, hr2 = r2 // HROW, r2 % HROW
        scores = scores_box[h]
        for j in range(2):
            r = hr2 + j
            nc.tensor.matmul(
                scores[:],
                lhsT=vt_sb[:, 0, (HROW - 1) - r:(HROW - 1) - r + HROW],
                rhs=esc[:, j, :],
                start=first_v[h],
                stop=(r == HROW - 1),
                skip_group_check=True)
            first_v[h] = False
        if r2 + 2 == HROW:
            emit_exp(0)
        elif r2 == HROW:
            emit_normalize(0)

    pending = []
    for b in range(BPC):
        # stream the 4 k-chunks as full [128, 4096] DMAs (8 KB descriptor
        # rows); batch 0's were issued above.
        if b == 0:
            xt = xt0
        else:
            xt = []
            for c in range(NK):
                t = xpool.tile([128, S], F16, tag="x", name=f"x{c}")
                xt.append(t)
            for c in range(NK):
                nc.sync.dma_start(xt[c][:], x_ap[b, c * 128:(c + 1) * 128, :])

        for g in range(NG):
            es_pair = []
            for m in range(NM):
                eps = epsum.tile([128, 2, ST], F32, tag="ep")
                for c in range(NK):
                    for j in range(2):
                        nc.tensor.matmul(
                            eps[:, j, :],
                            lhsT=wt_sb[:, c, m * 128:(m + 1) * 128],
                            rhs=xt[c][:, (2 * g + j) * ST:(2 * g + j + 1) * ST],
                            start=(c == 0), stop=(c == NK - 1))
                if b == 0 and g == 0 and m == 0:
                    # bias matmuls ride here: after a group of E-matmuls
                    # (so the PE has work while the SWDGE w3/ht loads
                    # land) but before the first tanh that reads bias_sb
                    emit_bias()
                    scores_box[0] = scpsum.tile([HROW, ST], F32, tag="scA",
                                                name="scoresA")
                    scores_box[1] = scpsum.tile([HROW, ST], F32, tag="scB",
                                                name="scoresB")
                    nc.vector.memset(sums[:], 0.0)
                    nc.vector.memset(inv[:], 0.0)
                es = esb.tile([128, 2, ST], F16, tag="es")
                nc.scalar.activation(es[:], eps[:],
                                     TANH, bias=bias_sb[:, m, b:b + 1])
                es_pair.append(es)

            esc = ecb.tile([128, 2, ST], F16, tag="ec")
            tmp = ecb.tile([128, 2, ST], F16, tag="ec2")
            nc.vector.tensor_scalar_mul(esc[:], es_pair[0][:], vp_sb[:, 0:1])
            nc.vector.tensor_scalar_mul(tmp[:], es_pair[1][:], vp_sb[:, 1:2])
            nc.vector.tensor_tensor(esc[:], esc[:], tmp[:],
                                    op=mybir.AluOpType.add)
            pending.append((b * NS + 2 * g, esc))
            if len(pending) > 2:
                emit_v(pending.pop(0))

    for pend in pending:
        emit_v(pend)

    # ---- tail: exp + normalize + output DMA for half B only (half A
    # already went out mid-kernel)
    emit_exp(1)
    emit_normalize(1)


_CACHED = None


def _build():
    global _CACHED
    if _CACHED is not None:
        return _CACHED
    nc = bacc.Bacc("TRN2", target_bir_lowering=False, debug=False,
                   num_devices=NCORES)
    x = nc.dram_tensor("x", (BPC, 2 * H, S), F16, kind="ExternalInput").ap()
    wt = nc.dram_tensor("wt", (128, NK, H), F16, kind="ExternalInput").ap()
    w3t = nc.dram_tensor("w3t", (128, 2, H), F32R, kind="ExternalInput").ap()
    vt = nc.dram_tensor("vt", (128, 2, 2 * HROW - 1), F16, kind="ExternalInput").ap()
    ht = nc.dram_tensor("ht", (128, 2, BPC), F32R, kind="ExternalInput").ap()
    blk = nc.dram_tensor("blk", (128, BPC), F16, kind="ExternalInput").ap()
    blkT = nc.dram_tensor("blkT", (128, NROW), F16, kind="ExternalInput").ap()
    vp = nc.dram_tensor("vp", (128, 2), F32, kind="ExternalInput").ap()
    out = nc.dram_tensor("out", (BPC, S), F32, kind="ExternalOutput").ap()

    with tile.TileContext(nc) as tc:
        _attn_kernel(tc, out, x, wt, w3t, vt, ht, blk, blkT, vp)
    nc.compile()
    _CACHED = nc
    return nc


def _chunk_major(a: np.ndarray) -> np.ndarray:
    """[C*128, F] -> [128, C, F] so partition p holds rows {p, 128+p, ...}."""
    c = a.shape[0] // 128
    return np.ascontiguousarray(a.reshape(c, 128, -1).transpose(1, 0, 2))


def kernel(static_enc, dynamic_enc, decoder_hidden, v, W, *, _trace=False,
           **trace_kwargs):
    static_enc = np.asarray(static_enc, dtype=np.float16)
    dynamic_enc = np.asarray(dynamic_enc, dtype=np.float16)
    decoder_hidden = np.ascontiguousarray(decoder_hidden, dtype=np.float32)
    v = np.ascontiguousarray(v, dtype=np.float32)
    W = np.ascontiguousarray(W, dtype=np.float32)

    nc = _build()

    xcat = np.concatenate([static_enc, dynamic_enc], axis=1)  # [B, 2H, S]
    wt = _chunk_major(np.concatenate([W[:, :H].T, W[:, H:2 * H].T],
                                     axis=0).astype(np.float16))
    w3t = _chunk_major(np.ascontiguousarray(W[:, 2 * H:].T))
    # vt_ext[p, c, :] = [0]*31 ++ [v_c[p]] ++ [0]*31 ; lhsT window starting
    # at (HROW-1)-r puts v at output column r of its half, zeros elsewhere.
    vt_ext = np.zeros((128, 2, 2 * HROW - 1), dtype=np.float16)
    vt_ext[:, :, HROW - 1] = 1.0  # ones window: plain partition sum
    vp = np.ascontiguousarray(v.reshape(2, 128).T.astype(np.float32))
    blk = np.zeros((128, BPC), dtype=np.float16)
    for r in range(NROW):
        blk[r, r // NS] = 1.0
    # blkT[p, r] = 1 iff inv at partition p feeds score row r, per half:
    # half A uses rows/partitions 0-31, half B rows/partitions 32-63
    blkT = np.zeros((128, NROW), dtype=np.float16)
    for r in range(NROW):
        h = r // (NROW // 2)
        blkT[32 * h + (r - 32 * h) // NS, r] = 1.0
    in_maps = []
    for i in range(NCORES):
        sl = slice(i * BPC, (i + 1) * BPC)
        ht = _chunk_major(np.ascontiguousarray(decoder_hidden[sl].T))
        in_maps.append({
            "x": xcat[sl],
            "wt": wt, "w3t": w3t, "vt": vt_ext, "ht": ht,
            "blk": blk, "blkT": blkT, "vp": vp,
        })

    res = run_bass_kernel_spmd(nc, in_maps, core_ids=list(range(NCORES)),
                               trace=_trace, **trace_kwargs)
    kernel.last_result = res
    return np.concatenate([res.results[i]["out"] for i in range(NCORES)], axis=0)


kernel.last_result = None
